# revision 16
# baseline (speedup 1.0000x reference)
"""ContrastLoss (InfoNCE-style) Trainium2 kernel, data-parallel over batch on 8 cores.

Math (per sample b):
    s[i,j] = (tmap[b,i,j] . qhat[b]) / ||tmap[b,i,j]||        (qhat = normalized pos_query)
    e = exp(s); num = sum(e * pos_mask); den = num + sum(e * neg_mask)
    li = -log(num / (den + EPS)); loss = mean(li over valid samples)

Device strategy (v2, PE-centric, bf16):
  Host pre-transposes tmap to (H, cells) bf16 per sample, so H sits on SBUF
  partitions (2 halves of 128) and cells on the free dim. Per sample:
    - dot(t, qhat) and sumsq(t) both become partition-dim contractions on the
      TensorEngine. Squares come from a DVE bf16 tensor_tensor (2x perf mode).
    - Zero-padded stationaries [qhat_half | 0] and [0 | ones] accumulate into
      ONE (2, 2048) PSUM tile per phase: row 0 = dot, row 1 = sumsq.
  One small DMA per phase evacuates PSUM into a (128, 32) layout (cells
  across partitions; dot in cols 0-15, ssq in cols 16-31) where the epilogue
  runs 128-wide: 1/||t|| = exp(-0.5*ln(ssq)) on ScalarE, e = exp(dot/||t||),
  masked pos/neg sums via DVE tensor ops. 128x16 partial sums per phase ship
  to the host for the final tiny reduction (-log, valid masking, mean).

  bf16 halves HBM traffic (memory-bound regime) and costs ~1e-3 absolute
  error in s, far inside the 2e-2 gate.
"""

import numpy as np
import ml_dtypes

import concourse.bacc as bacc
import concourse.tile as tile
from concourse import mybir
from concourse.bass_utils import run_bass_kernel_spmd
from concourse.hw_specs import get_activation_tables as _real_gat

_ACT_SET = "natural_log_exp_and_others"  # contains ln, exp


def _patched_gat(arch):
    """Force every activation to resolve to the one set containing all our
    functions (ln/exp), avoiding per-phase table-set thrashing (~2.7us per
    reload). Indices into act_info.json are preserved."""
    tabs = _real_gat(arch)
    return {k: (v if k == _ACT_SET else set()) for k, v in tabs.items()}


bacc.get_activation_tables = _patched_gat

N_CORES = 8
B, S, H = 32, 64, 256
BS = B // N_CORES          # samples per core (4)
SH = S * S                 # cells per sample (4096)
HC = SH // 2               # cells per phase = cell-half (2048)
BLK = 512                  # matmul moving-block size
NPH = BS * 2               # phases per core (8): (sample, cell-half)
EPS = 1e-8

BF16 = ml_dtypes.bfloat16

_NC_CACHE = {}


def _build_nc(loop_reps=0):
    """loop_reps=0: straight-line kernel. loop_reps=N>0: wrap the whole body
    in a tc.For_i loop that re-runs it N times (identical data; used only for
    differential wall-clock timing of the device execution)."""
    A = mybir.ActivationFunctionType
    OP = mybir.AluOpType
    dt = mybir.dt

    nc = bacc.Bacc(
        "TRN2",
        target_bir_lowering=False,
        debug=False,
        enable_asserts=False,
        num_devices=N_CORES,
    )

    t_in = nc.dram_tensor("t_in", [2 * BS, 128, SH], dt.bfloat16, kind="ExternalInput").ap()
    q_in = nc.dram_tensor("q_in", [128, 4 * BS], dt.bfloat16, kind="ExternalInput").ap()
    pm_in = nc.dram_tensor("pm_in", [128, NPH * 16], dt.float32, kind="ExternalInput").ap()
    nm_in = nc.dram_tensor("nm_in", [128, NPH * 16], dt.float32, kind="ExternalInput").ap()
    parts = nc.dram_tensor("parts", [128, 2 * NPH], dt.float32, kind="ExternalOutput").ap()

    with tile.TileContext(nc) as tc:
        with (
            tc.tile_pool(name="small", bufs=1) as spool,
            tc.tile_pool(name="tmaps", bufs=3) as tpool,
            tc.tile_pool(name="sqs", bufs=3) as sqpool,
            tc.tile_pool(name="psums", bufs=2, space="PSUM") as pspool,
            tc.tile_pool(name="stats", bufs=4) as stpool,
        ):
            qsb = spool.tile([128, 4 * BS], dt.bfloat16, tag="qsb")
            nc.sync.dma_start(out=qsb[:], in_=q_in[:])
            # masks are only needed at the first epilogue (~8us in) - load
            # them on a different DGE engine so they don't delay t loads.
            pmsb = spool.tile([128, NPH * 16], dt.float32, tag="pmsb")
            nc.sync.dma_start(out=pmsb[:], in_=pm_in[:])
            nmsb = spool.tile([128, NPH * 16], dt.float32, tag="nmsb")
            nc.sync.dma_start(out=nmsb[:], in_=nm_in[:])
            # [0 | ones] stationary for the sumsq stream
            onz = spool.tile([128, 2], dt.bfloat16, tag="onz")
            nc.vector.memset(onz[:], 0.0)
            nc.vector.memset(onz[:, 1:2], 1.0)
            npart = spool.tile([128, 2 * NPH], dt.float32, tag="npart")

            import contextlib
            loop_cm = tc.For_i(0, loop_reps, 1) if loop_reps else contextlib.nullcontext()
            with loop_cm:
                _emit_body(nc, tc, tpool, sqpool, pspool, stpool,
                           t_in, qsb, pmsb, nmsb, onz, npart, A, OP, dt)

            nc.sync.dma_start(out=parts[:], in_=npart[:])

    nc.compile()
    return nc


def _emit_body(nc, tc, tpool, sqpool, pspool, stpool,
               t_in, qsb, pmsb, nmsb, onz, npart, A, OP, dt):
    pending = None
    for s in range(BS):
        # Load t in (h, cell-half) quarters so phase 0's matmuls can start
        # after ~1/4 of the sample's DMA instead of all of it.
        th = [[None, None], [None, None]]   # [h][ch]
        sq = [[None, None], [None, None]]
        for ch in range(2):
            for h in range(2):
                t_tile = tpool.tile([128, HC], dt.bfloat16, tag=f"t{h}{ch}")
                nc.sync.dma_start(
                    out=t_tile[:],
                    in_=t_in[2 * s + h][:, ch * HC:(ch + 1) * HC],
                )
                th[h][ch] = t_tile
                sq_tile = sqpool.tile([128, HC], dt.bfloat16, tag=f"sq{h}{ch}")
                nc.vector.tensor_tensor(out=sq_tile[:], in0=t_tile[:],
                                        in1=t_tile[:], op=OP.mult)
                sq[h][ch] = sq_tile

        for ch in range(2):
            ph = 2 * s + ch
            last = (ph == NPH - 1)
            qparts = 1
            qw = HC // qparts
            ests2 = []
            for qp in range(qparts):
                ps = pspool.tile([2, qw], dt.float32, tag="ps")
                for h in range(2):
                    for blk in range(qw // BLK):
                        cs = slice(qp * qw + blk * BLK, qp * qw + (blk + 1) * BLK)
                        ob = slice(blk * BLK, (blk + 1) * BLK)
                        nc.tensor.matmul(
                            ps[:, ob],
                            qsb[:, 4 * s + 2 * h:4 * s + 2 * h + 2],
                            th[h][ch][:, cs],
                            start=(h == 0),
                            stop=False,
                        )
                for h in range(2):
                    for blk in range(qw // BLK):
                        cs = slice(qp * qw + blk * BLK, qp * qw + (blk + 1) * BLK)
                        ob = slice(blk * BLK, (blk + 1) * BLK)
                        nc.tensor.matmul(
                            ps[:, ob],
                            onz[:],
                            sq[h][ch][:, cs],
                            start=False,
                            stop=(h == 1),
                        )
                ests2.append(ps)

            # Evacuate PSUM: engine copy -> SBUF (PE has no PSUM read port
            # for DMA), then relayout DMAs into (128, 32) with dot in cols
            # 0-15, ssq in cols 16-31: out[(r, p, j)] = in[r*2048 + p*16 + j].
            est = stpool.tile([128, 32], dt.float32, tag="est")
            for qp in range(qparts):
                ps = ests2[qp]
                est2 = stpool.tile([2, qw], dt.float32, tag=f"est2{qp}" if last else "est2")
                if last:
                    nc.vector.tensor_copy(est2[:], ps[:])
                else:
                    nc.scalar.copy(est2[:], ps[:])
                # quarter qp covers cells [qp*qw, (qp+1)*qw): partitions
                # p = c // 16 in [qp*qw//16, ...): rows qp*64..qp*64+63 when split.
                p0 = qp * (qw // 16)
                p1 = p0 + (qw // 16)
                nc.sync.dma_start(out=est[p0:p1, 16:32], in_=est2[1:2, :])
                nc.sync.dma_start(out=est[p0:p1, 0:16], in_=est2[0:1, :])

            # Defer the epilogue by one phase: ScalarE's exec queue is a
            # strict FIFO, so a Ln head-waiting on the est DMA would block
            # the next phase's PSUM-releasing copy and stall the PE.
            if pending is not None:
                _emit_epilogue(nc, stpool, pmsb, nmsb, npart, A, OP, dt, *pending)
            pending = (ph, est)
    _emit_epilogue(nc, stpool, pmsb, nmsb, npart, A, OP, dt, *pending)


def _emit_epilogue(nc, stpool, pmsb, nmsb, npart, A, OP, dt, ph, est):
    dsb = est[:, 0:16]
    ssb = est[:, 16:32]
    # invn = exp(-0.5*ln(ssq)); e = exp(dot*invn)
    lnb = stpool.tile([128, 16], dt.float32, tag="lnb")
    nc.scalar.activation(lnb[:], ssb, A.Ln)
    invn = stpool.tile([128, 16], dt.float32, tag="invn")
    nc.scalar.activation(invn[:], lnb[:], A.Exp, scale=-0.5)
    sb = stpool.tile([128, 16], dt.float32, tag="sb")
    nc.vector.tensor_mul(sb[:], dsb, invn[:])
    eb = stpool.tile([128, 16], dt.float32, tag="eb")
    nc.scalar.activation(eb[:], sb[:], A.Exp)
    msk_scr = stpool.tile([128, 16], dt.float32, tag="msk")
    nc.vector.scalar_tensor_tensor(
        out=msk_scr[:], in0=eb[:], scalar=0.0,
        in1=pmsb[:, ph * 16:(ph + 1) * 16],
        op0=OP.bypass, op1=OP.mult,
        accum_out=npart[:, 2 * ph:2 * ph + 1],
    )
    msk_scr2 = stpool.tile([128, 16], dt.float32, tag="msk2")
    nc.vector.scalar_tensor_tensor(
        out=msk_scr2[:], in0=eb[:], scalar=0.0,
        in1=nmsb[:, ph * 16:(ph + 1) * 16],
        op0=OP.bypass, op1=OP.mult,
        accum_out=npart[:, 2 * ph + 1:2 * ph + 2],
    )


def get_nc(loop_reps=0):
    key = ("nc", loop_reps)
    if key not in _NC_CACHE:
        _NC_CACHE[key] = _build_nc(loop_reps)
    return _NC_CACHE[key]


def _permute_mask(m):
    """(BS, S, S) bool -> (128, NPH*16) f32 matching the evac layout.

    Phase ph = 2s+ch covers cells [ch*2048, (ch+1)*2048) of sample s.
    Partition p holds cells ch*2048 + p*16 + j, at column ph*16 + j."""
    x = m.reshape(BS, 2, 128, 16).astype(np.float32)   # (s, ch, p, j)
    return np.ascontiguousarray(x.transpose(2, 0, 1, 3)).reshape(128, NPH * 16)


def make_in_maps(pos_query, tmap, mask2d_pos, mask2d_neg):
    pq = np.asarray(pos_query, dtype=np.float32)
    tm = np.ascontiguousarray(np.asarray(tmap, dtype=np.float32))
    mp = np.asarray(mask2d_pos).astype(bool)
    mn = np.asarray(mask2d_neg).astype(bool)

    qn = np.sqrt(np.sum(pq * pq, axis=-1, keepdims=True, dtype=np.float32))
    qhat = (pq / (qn + np.float32(EPS))).astype(np.float32)

    in_maps = []
    for c in range(N_CORES):
        sl = slice(c * BS, (c + 1) * BS)
        # (BS, SH, H) -> (BS, H, SH) -> (2*BS, 128, SH) bf16
        tt = tm[sl].reshape(BS, SH, H).transpose(0, 2, 1)
        t_dev = np.ascontiguousarray(tt).reshape(2 * BS, 128, SH).astype(BF16)
        # q_in[p, 4s+2h] = qhat[b0+s, h*128+p]; odd columns zero.
        q_dev = np.zeros((128, 4 * BS), dtype=BF16)
        qr = qhat[sl].reshape(BS, 2, 128)                # (s, h, p)
        q_dev[:, 0::2] = qr.transpose(2, 0, 1).reshape(128, 2 * BS)
        in_maps.append({
            "t_in": t_dev,
            "q_in": q_dev,
            "pm_in": _permute_mask(mp[sl]),
            "nm_in": _permute_mask(mn[sl]),
        })
    return in_maps, mp, mn


def finish(parts_per_core, mp, mn):
    """parts_per_core: list of (128, 2*NPH) arrays -> scalar loss (np.float32)."""
    num = np.zeros(B, np.float32)
    neg = np.zeros(B, np.float32)
    for c in range(N_CORES):
        p = parts_per_core[c].reshape(128, BS, 2, 2)     # (p, s, ch, k)
        sums = p.sum(axis=(0, 2), dtype=np.float32)      # (s, k)
        for s in range(BS):
            num[c * BS + s] = sums[s, 0]
            neg[c * BS + s] = sums[s, 1]
    den = num + neg
    with np.errstate(divide="ignore", invalid="ignore", over="ignore"):
        li = -np.log(num / (den + np.float32(EPS)))
    valid = mp.any(axis=(1, 2)) & mn.any(axis=(1, 2))
    n_valid = max(int(valid.sum()), 1)
    loss = np.where(valid, li, np.float32(0.0)).sum(dtype=np.float32) / np.float32(n_valid)
    return np.asarray(loss, dtype=np.float32)


def kernel(pos_query, tmap, mask2d_pos, mask2d_neg):
    in_maps, mp, mn = make_in_maps(pos_query, tmap, mask2d_pos, mask2d_neg)
    nc = get_nc()
    res = run_bass_kernel_spmd(nc, in_maps, list(range(N_CORES)))
    parts_per_core = [res.results[c]["parts"] for c in range(N_CORES)]
    return finish(parts_per_core, mp, mn)


if __name__ == "__main__":
    # Smoke test with random data (no reference).
    rng = np.random.default_rng(0)
    inputs = {
        "pos_query": rng.standard_normal((B, H), dtype=np.float32),
        "tmap": rng.standard_normal((B, S, S, H), dtype=np.float32),
        "mask2d_pos": rng.random((B, S, S)) < 0.05,
        "mask2d_neg": (rng.random((B, S, S)) >= 0.05) & (rng.random((B, S, S)) < 0.35),
    }
    print(kernel(**inputs))


# revision 17
# speedup vs baseline: 1.9663x; 1.9663x over previous
"""ContrastLoss (InfoNCE-style) Trainium2 kernel, data-parallel over batch on 8 cores.

Math (per sample b):
    s[i,j] = (tmap[b,i,j] . qhat[b]) / ||tmap[b,i,j]||        (qhat = normalized pos_query)
    e = exp(s); num = sum(e * pos_mask); den = num + sum(e * neg_mask)
    li = -log(num / (den + EPS)); loss = mean(li over valid samples)

Device strategy (v2, PE-centric, bf16):
  Host pre-transposes tmap to (H, cells) bf16 per sample, so H sits on SBUF
  partitions (2 halves of 128) and cells on the free dim. Per sample:
    - dot(t, qhat) and sumsq(t) both become partition-dim contractions on the
      TensorEngine. Squares come from a DVE bf16 tensor_tensor (2x perf mode).
    - Zero-padded stationaries [qhat_half | 0] and [0 | ones] accumulate into
      ONE (2, 2048) PSUM tile per phase: row 0 = dot, row 1 = sumsq.
  One small DMA per phase evacuates PSUM into a (128, 32) layout (cells
  across partitions; dot in cols 0-15, ssq in cols 16-31) where the epilogue
  runs 128-wide: 1/||t|| = exp(-0.5*ln(ssq)) on ScalarE, e = exp(dot/||t||),
  masked pos/neg sums via DVE tensor ops. 128x16 partial sums per phase ship
  to the host for the final tiny reduction (-log, valid masking, mean).

  bf16 halves HBM traffic (memory-bound regime) and costs ~1e-3 absolute
  error in s, far inside the 2e-2 gate.
"""

import numpy as np
import ml_dtypes

import concourse.bacc as bacc
import concourse.tile as tile
from concourse import mybir
from concourse.bass_utils import run_bass_kernel_spmd
from concourse.hw_specs import get_activation_tables as _real_gat

_ACT_SET = "natural_log_exp_and_others"  # contains ln, exp


def _patched_gat(arch):
    """Force every activation to resolve to the one set containing all our
    functions (ln/exp), avoiding per-phase table-set thrashing (~2.7us per
    reload). Indices into act_info.json are preserved."""
    tabs = _real_gat(arch)
    return {k: (v if k == _ACT_SET else set()) for k, v in tabs.items()}


bacc.get_activation_tables = _patched_gat

N_CORES = 8
B, S, H = 32, 64, 256
BS = B // N_CORES          # samples per core (4)
SH = S * S                 # cells per sample (4096)
HC = SH // 2               # cells per phase = cell-half (2048)
BLK = 512                  # matmul moving-block size
NPH = BS * 2               # phases per core (8): (sample, cell-half)
EPS = 1e-8

BF16 = ml_dtypes.bfloat16

_NC_CACHE = {}


def _build_nc(loop_reps=0):
    """loop_reps=0: straight-line kernel. loop_reps=N>0: wrap the whole body
    in a tc.For_i loop that re-runs it N times (identical data; used only for
    differential wall-clock timing of the device execution)."""
    A = mybir.ActivationFunctionType
    OP = mybir.AluOpType
    dt = mybir.dt

    nc = bacc.Bacc(
        "TRN2",
        target_bir_lowering=False,
        debug=False,
        enable_asserts=False,
        num_devices=N_CORES,
    )

    t_in = nc.dram_tensor("t_in", [2 * BS, 128, SH], dt.bfloat16, kind="ExternalInput").ap()
    q_in = nc.dram_tensor("q_in", [128, 4 * BS], dt.bfloat16, kind="ExternalInput").ap()
    pm_in = nc.dram_tensor("pm_in", [128, NPH * 16], dt.float32, kind="ExternalInput").ap()
    nm_in = nc.dram_tensor("nm_in", [128, NPH * 16], dt.float32, kind="ExternalInput").ap()
    parts = nc.dram_tensor("parts", [128, 2 * NPH], dt.float32, kind="ExternalOutput").ap()

    with tile.TileContext(nc) as tc:
        with (
            tc.tile_pool(name="small", bufs=1) as spool,
            tc.tile_pool(name="tmaps", bufs=3) as tpool,
            tc.tile_pool(name="sqs", bufs=3) as sqpool,
            tc.tile_pool(name="psums", bufs=2, space="PSUM") as pspool,
            tc.tile_pool(name="stats", bufs=4) as stpool,
        ):
            qsb = spool.tile([128, 4 * BS], dt.bfloat16, tag="qsb")
            nc.sync.dma_start(out=qsb[:], in_=q_in[:])
            # masks are only needed at the first epilogue (~8us in) - load
            # them on a different DGE engine so they don't delay t loads.
            pmsb = spool.tile([128, NPH * 16], dt.float32, tag="pmsb")
            nc.sync.dma_start(out=pmsb[:], in_=pm_in[:])
            nmsb = spool.tile([128, NPH * 16], dt.float32, tag="nmsb")
            nc.sync.dma_start(out=nmsb[:], in_=nm_in[:])
            # [0 | ones] stationary for the sumsq stream
            onz = spool.tile([128, 2], dt.bfloat16, tag="onz")
            nc.vector.memset(onz[:], 0.0)
            nc.vector.memset(onz[:, 1:2], 1.0)
            npart = spool.tile([128, 2 * NPH], dt.float32, tag="npart")

            import contextlib
            loop_cm = tc.For_i(0, loop_reps, 1) if loop_reps else contextlib.nullcontext()
            with loop_cm:
                _emit_body(nc, tc, tpool, sqpool, pspool, stpool,
                           t_in, qsb, pmsb, nmsb, onz, npart, A, OP, dt)

            nc.sync.dma_start(out=parts[:], in_=npart[:])

    nc.compile()
    return nc


def _emit_body(nc, tc, tpool, sqpool, pspool, stpool,
               t_in, qsb, pmsb, nmsb, onz, npart, A, OP, dt):
    for s in range(BS):
        # Load t in (h, cell-half) quarters so phase 0's matmuls can start
        # after ~1/4 of the sample's DMA instead of all of it.
        th = [[None, None], [None, None]]   # [h][ch]
        sq = [[None, None], [None, None]]
        for ch in range(2):
            for h in range(2):
                t_tile = tpool.tile([128, HC], dt.bfloat16, tag=f"t{h}{ch}")
                nc.sync.dma_start(
                    out=t_tile[:],
                    in_=t_in[2 * s + h][:, ch * HC:(ch + 1) * HC],
                )
                th[h][ch] = t_tile
                sq_tile = sqpool.tile([128, HC], dt.bfloat16, tag=f"sq{h}{ch}")
                nc.vector.tensor_tensor(out=sq_tile[:], in0=t_tile[:],
                                        in1=t_tile[:], op=OP.mult)
                sq[h][ch] = sq_tile

        for ch in range(2):
            ph = 2 * s + ch
            last = (ph == NPH - 1)
            qparts = 1
            qw = HC // qparts
            ests2 = []
            for qp in range(qparts):
                ps = pspool.tile([2, qw], dt.float32, tag="ps")
                for h in range(2):
                    for blk in range(qw // BLK):
                        cs = slice(qp * qw + blk * BLK, qp * qw + (blk + 1) * BLK)
                        ob = slice(blk * BLK, (blk + 1) * BLK)
                        nc.tensor.matmul(
                            ps[:, ob],
                            qsb[:, 4 * s + 2 * h:4 * s + 2 * h + 2],
                            th[h][ch][:, cs],
                            start=(h == 0),
                            stop=False,
                        )
                for h in range(2):
                    for blk in range(qw // BLK):
                        cs = slice(qp * qw + blk * BLK, qp * qw + (blk + 1) * BLK)
                        ob = slice(blk * BLK, (blk + 1) * BLK)
                        nc.tensor.matmul(
                            ps[:, ob],
                            onz[:],
                            sq[h][ch][:, cs],
                            start=False,
                            stop=(h == 1),
                        )
                ests2.append(ps)

            # Evacuate PSUM: engine copy -> SBUF (PE has no PSUM read port
            # for DMA), then relayout DMAs into (128, 32) with dot in cols
            # 0-15, ssq in cols 16-31: out[(r, p, j)] = in[r*2048 + p*16 + j].
            est = stpool.tile([128, 32], dt.float32, tag="est")
            for qp in range(qparts):
                ps = ests2[qp]
                est2 = stpool.tile([2, qw], dt.float32, tag="est2")
                nc.scalar.copy(est2[:], ps[:])
                p0 = qp * (qw // 16)
                p1 = p0 + (qw // 16)
                nc.sync.dma_start(out=est[p0:p1, 16:32], in_=est2[1:2, :])
                nc.sync.dma_start(out=est[p0:p1, 0:16], in_=est2[0:1, :])

            _emit_epilogue(nc, stpool, pmsb, nmsb, npart, A, OP, dt, ph, est)


def _emit_epilogue(nc, stpool, pmsb, nmsb, npart, A, OP, dt, ph, est):
    dsb = est[:, 0:16]
    ssb = est[:, 16:32]
    # invn = exp(-0.5*ln(ssq)); e = exp(dot*invn)
    lnb = stpool.tile([128, 16], dt.float32, tag="lnb")
    nc.scalar.activation(lnb[:], ssb, A.Ln)
    invn = stpool.tile([128, 16], dt.float32, tag="invn")
    nc.scalar.activation(invn[:], lnb[:], A.Exp, scale=-0.5)
    sb = stpool.tile([128, 16], dt.float32, tag="sb")
    nc.vector.tensor_mul(sb[:], dsb, invn[:])
    eb = stpool.tile([128, 16], dt.float32, tag="eb")
    nc.scalar.activation(eb[:], sb[:], A.Exp)
    msk_scr = stpool.tile([128, 16], dt.float32, tag="msk")
    nc.vector.scalar_tensor_tensor(
        out=msk_scr[:], in0=eb[:], scalar=0.0,
        in1=pmsb[:, ph * 16:(ph + 1) * 16],
        op0=OP.bypass, op1=OP.mult,
        accum_out=npart[:, 2 * ph:2 * ph + 1],
    )
    msk_scr2 = stpool.tile([128, 16], dt.float32, tag="msk2")
    nc.vector.scalar_tensor_tensor(
        out=msk_scr2[:], in0=eb[:], scalar=0.0,
        in1=nmsb[:, ph * 16:(ph + 1) * 16],
        op0=OP.bypass, op1=OP.mult,
        accum_out=npart[:, 2 * ph + 1:2 * ph + 2],
    )


def get_nc(loop_reps=0):
    key = ("nc", loop_reps)
    if key not in _NC_CACHE:
        _NC_CACHE[key] = _build_nc(loop_reps)
    return _NC_CACHE[key]


def _permute_mask(m):
    """(BS, S, S) bool -> (128, NPH*16) f32 matching the evac layout.

    Phase ph = 2s+ch covers cells [ch*2048, (ch+1)*2048) of sample s.
    Partition p holds cells ch*2048 + p*16 + j, at column ph*16 + j."""
    x = m.reshape(BS, 2, 128, 16).astype(np.float32)   # (s, ch, p, j)
    return np.ascontiguousarray(x.transpose(2, 0, 1, 3)).reshape(128, NPH * 16)


def make_in_maps(pos_query, tmap, mask2d_pos, mask2d_neg):
    pq = np.asarray(pos_query, dtype=np.float32)
    tm = np.ascontiguousarray(np.asarray(tmap, dtype=np.float32))
    mp = np.asarray(mask2d_pos).astype(bool)
    mn = np.asarray(mask2d_neg).astype(bool)

    qn = np.sqrt(np.sum(pq * pq, axis=-1, keepdims=True, dtype=np.float32))
    qhat = (pq / (qn + np.float32(EPS))).astype(np.float32)

    in_maps = []
    for c in range(N_CORES):
        sl = slice(c * BS, (c + 1) * BS)
        # (BS, SH, H) -> (BS, H, SH) -> (2*BS, 128, SH) bf16
        tt = tm[sl].reshape(BS, SH, H).transpose(0, 2, 1)
        t_dev = np.ascontiguousarray(tt).reshape(2 * BS, 128, SH).astype(BF16)
        # q_in[p, 4s+2h] = qhat[b0+s, h*128+p]; odd columns zero.
        q_dev = np.zeros((128, 4 * BS), dtype=BF16)
        qr = qhat[sl].reshape(BS, 2, 128)                # (s, h, p)
        q_dev[:, 0::2] = qr.transpose(2, 0, 1).reshape(128, 2 * BS)
        in_maps.append({
            "t_in": t_dev,
            "q_in": q_dev,
            "pm_in": _permute_mask(mp[sl]),
            "nm_in": _permute_mask(mn[sl]),
        })
    return in_maps, mp, mn


def finish(parts_per_core, mp, mn):
    """parts_per_core: list of (128, 2*NPH) arrays -> scalar loss (np.float32)."""
    num = np.zeros(B, np.float32)
    neg = np.zeros(B, np.float32)
    for c in range(N_CORES):
        p = parts_per_core[c].reshape(128, BS, 2, 2)     # (p, s, ch, k)
        sums = p.sum(axis=(0, 2), dtype=np.float32)      # (s, k)
        for s in range(BS):
            num[c * BS + s] = sums[s, 0]
            neg[c * BS + s] = sums[s, 1]
    den = num + neg
    with np.errstate(divide="ignore", invalid="ignore", over="ignore"):
        li = -np.log(num / (den + np.float32(EPS)))
    valid = mp.any(axis=(1, 2)) & mn.any(axis=(1, 2))
    n_valid = max(int(valid.sum()), 1)
    loss = np.where(valid, li, np.float32(0.0)).sum(dtype=np.float32) / np.float32(n_valid)
    return np.asarray(loss, dtype=np.float32)


def kernel(pos_query, tmap, mask2d_pos, mask2d_neg):
    in_maps, mp, mn = make_in_maps(pos_query, tmap, mask2d_pos, mask2d_neg)
    nc = get_nc()
    res = run_bass_kernel_spmd(nc, in_maps, list(range(N_CORES)))
    parts_per_core = [res.results[c]["parts"] for c in range(N_CORES)]
    return finish(parts_per_core, mp, mn)


if __name__ == "__main__":
    # Smoke test with random data (no reference).
    rng = np.random.default_rng(0)
    inputs = {
        "pos_query": rng.standard_normal((B, H), dtype=np.float32),
        "tmap": rng.standard_normal((B, S, S, H), dtype=np.float32),
        "mask2d_pos": rng.random((B, S, S)) < 0.05,
        "mask2d_neg": (rng.random((B, S, S)) >= 0.05) & (rng.random((B, S, S)) < 0.35),
    }
    print(kernel(**inputs))
